# revision 2
# baseline (speedup 1.0000x reference)
"""Trainium2 Bass kernel v2 for nn_CustomLayerMKM: y = x @ (sum_k kron(Bk, Ak)).T + b.

Data-parallel over B across 8 cores (512 rows each). Per core:

  stage 1 (PE): per quarter q (128 b-rows), per i-block t: one matmul with
    lhsT = xT-block (stationary) and rhs = patAcat [128, 384] (all 3 factors'
    A-patterns concatenated) -> psum [128 b, 384]. Blocks grouped in triples
    sharing a 3-bank psum tile so evictions are few and fat.
  eviction (DVE/ACT alternating): psum -> U_q [128 b, 12288] bf16 with a
    5D scatter placing col (k,u,r) of block t at (u*3+k)*128 + 4t + r, so
    every 128-col block of U_q is one stage-2 contraction block.
  corner-turn (xbar DMA, sync+scalar queues): U_q halves [128, 6144] ->
    V_half [128 v, 96 blk, 256 b] b-slices.  12.6MB total.
  stage 2 (PE): per b-half, per u-group: 3 matmuls (lhsT = patB'_k
    stationary, rhs = V[:, u*3+k, :], N=256) accumulate in psum ->
    y_T [o, b] evicted as bf16, DMAed out transposed.

Host: builds xT quarter-major bf16, the pattern matrices, reassembles
y from [half, c, u, b] chunks, adds bias in fp32.
"""

from contextlib import ExitStack

import numpy as np

P = 128
B_FULL, I_DIM, O_DIM = 4096, 4096, 4096
N_CORES = 8
B_SHARD = B_FULL // N_CORES          # 512
NQ = 4                               # quarters of 128 b-rows
TB = I_DIM // P                      # 32 i-blocks
NBLK = 96                            # (u, k) stage-2 blocks
# (m, n, f2, f1, H, G) per factor: A (m,n), B (f2,f1), H=m/32, G=128/n
FAC = [(64, 64, 64, 64, 2, 2), (128, 128, 32, 32, 4, 1), (32, 32, 128, 128, 1, 4)]


def build_nc(dump=False):
    import concourse.bass as bass
    import concourse.mybir as mybir
    import concourse.tile as tile
    from concourse import bacc

    BF16 = mybir.dt.bfloat16
    F32 = mybir.dt.float32

    nc = bacc.Bacc("TRN2", target_bir_lowering=False, debug=False,
                   num_devices=N_CORES)
    if dump:
        U0_ext = nc.dram_tensor("U0", [P, TB * 384], BF16,
                                kind="ExternalOutput").ap()
        V0_ext = nc.dram_tensor("V0", [P, NBLK, 256], BF16,
                                kind="ExternalOutput").ap()

    xT_ext = nc.dram_tensor("xT", [NQ, I_DIM, P], BF16,
                            kind="ExternalInput").ap()
    patA_ext = nc.dram_tensor("patA", [P, 384], BF16, kind="ExternalInput").ap()
    patB_ext = nc.dram_tensor("patB", [3, P, P], BF16, kind="ExternalInput").ap()
    # y stored transposed: [half][c][u][b]; host reassembles o = c*32 + u
    y_ext = nc.dram_tensor("y", [2, P, 32, 256], BF16, kind="ExternalOutput").ap()

    with tile.TileContext(nc) as tc, ExitStack() as ctx:
        const = ctx.enter_context(tc.tile_pool(name="const", bufs=1))
        ps = ctx.enter_context(tc.tile_pool(name="ps", bufs=2, space="PSUM"))
        xtp = ctx.enter_context(tc.tile_pool(name="xtp", bufs=2))
        up = ctx.enter_context(tc.tile_pool(name="up", bufs=2))
        vp = ctx.enter_context(tc.tile_pool(name="vp", bufs=1))
        yp = ctx.enter_context(tc.tile_pool(name="yp", bufs=2))

        patA_sb = const.tile([P, 384], BF16, tag="patA")
        patB_sb = const.tile([P, 3, P], BF16, tag="patB")

        # four V tiles (b-half x blk-half) so the two xbar ops of a quarter
        # write different tensors (no conservative WAW chaining)
        V = [[vp.tile([P, 48, 256], BF16, tag=f"V{h}{uh}", name=f"V{h}{uh}")
              for uh in range(2)] for h in range(2)]

        n_ev = [0]

        def evict(dst, src):
            eng = nc.vector.tensor_copy if n_ev[0] % 2 == 0 else nc.scalar.copy
            eng(dst, src)
            n_ev[0] += 1

        # ---------------- stage 1 + corner turn, per quarter ----------------
        # eviction engine pattern: DVE is ~1.5x faster per element -> 3:2 mix
        EV_PAT = "DADDADADADDADADA"
        xqs = {}

        def load_x(q, split=False):
            xq = xtp.tile([P, TB, P], BF16, tag="xq", name=f"xq{q}")
            xsrc = xT_ext[q].rearrange("(t p) b -> p t b", p=P, t=TB)
            if split:
                nc.sync.dma_start(xq[:, 0:16, :], xsrc[:, 0:16])
                nc.scalar.dma_start(xq[:, 16:32, :], xsrc[:, 16:32])
            else:
                nc.sync.dma_start(xq[:], xsrc)
            xqs[q] = xq

        load_x(0, split=True)
        nc.sync.dma_start(patA_sb[:], patA_ext[:])
        nc.sync.dma_start(patB_sb[:], patB_ext.rearrange("k p c -> p k c"))
        load_x(1)
        for q in range(NQ):
            xq = xqs[q]
            # prefetch the next quarter's x on sync BEFORE this quarter's
            # transposes so the DMA serialization order matches readiness
            if q + 2 < NQ:
                pass  # emitted after the xbars below (see end of loop)

            h, q2 = q // 2, q % 2
            U_q = up.tile([P, TB * 384], BF16, tag="U", name=f"U{q}")
            # dst scatter view: offset (u*3+k)*128 + 4t + r
            Uv = U_q.rearrange("p (u k t r) -> p t k u r", u=32, k=3, t=TB, r=4)
            for tp_ in range(16):           # t-pairs
                t0 = 2 * tp_
                pr = ps.tile([P, 1024], F32, tag="ps", bufs=4,
                             name=f"s1_{q}_{t0}")
                for tl in range(2):
                    nc.tensor.matmul(pr[:, 512 * tl:512 * tl + 384],
                                     xq[:, t0 + tl, :], patA_sb[:],
                                     start=True, stop=True)
                src = pr.rearrange("p (tl z) -> p tl z", tl=2, z=512)[
                    :, :, 0:384].rearrange(
                    "p tl (k u r) -> p tl k u r", k=3, u=32, r=4)
                if EV_PAT[n_ev[0] % 16] == "D":
                    nc.vector.tensor_copy(Uv[:, t0:t0 + 2], src)
                else:
                    # ACT ISA max 3 free dims: one instr per t of the pair
                    for tl in range(2):
                        nc.scalar.copy(Uv[:, t0 + tl], src[:, tl])
                n_ev[0] += 1

            if dump and q == 0:
                nc.sync.dma_start(U0_ext[:], U_q[:])
            nc.sync.dma_start_transpose(
                V[h][0][:, :, q2 * P:(q2 + 1) * P], U_q[:, 0:6144])
            nc.sync.dma_start_transpose(
                V[h][1][:, :, q2 * P:(q2 + 1) * P], U_q[:, 6144:12288])
            if q + 2 < NQ:
                load_x(q + 2)
        if dump:
            nc.sync.dma_start(V0_ext[:, 0:48], V[0][0][:])
            nc.sync.dma_start(V0_ext[:, 48:96], V[0][1][:])

        # ---------------- stage 2, per b-half ----------------
        for h in range(2):
            for c8 in range(8):            # chunks of 4 u-groups
                ych = yp.tile([P, 4, 256], BF16, tag="ych", name=f"y{h}_{c8}")
                for upair in range(2):
                    yps = ps.tile([P, 1024], F32, tag="ps", bufs=4,
                                  name=f"s2_{h}_{c8}_{upair}")[:, 0:512]
                    for ul in range(2):
                        u = c8 * 4 + upair * 2 + ul
                        for k in range(3):
                            nc.tensor.matmul(yps[:, 256 * ul:256 * (ul + 1)],
                                             patB_sb[:, k, :],
                                             V[h][u // 16][:, (u % 16) * 3 + k, :],
                                             start=(k == 0), stop=(k == 2))
                    evict(ych[:, 2 * upair:2 * upair + 2, :],
                          yps.rearrange("p (ul b) -> p ul b", ul=2, b=256))
                nc.scalar.dma_start(y_ext[h, :, c8 * 4:(c8 + 1) * 4, :], ych[:])

    nc.compile()
    return nc


_NC_CACHE = {}


def _build_patterns(As, Bs, bf16):
    patA = np.zeros((P, 384), np.float32)
    patB = np.zeros((3, P, P), np.float32)
    for k, (m, n, f2, f1, H, G) in enumerate(FAC):
        A, B = As[k], Bs[k]
        for u in range(32):
            for w in range(H):
                for g in range(G):
                    col = 128 * k + u * 4 + w * G + g
                    patA[g * n:(g + 1) * n, col] = A[u + 32 * w, :]
        for t in range(TB):
            for w in range(H):
                for g in range(G):
                    v = 4 * t + w * G + g
                    patB[k][v, np.arange(f2) * H + w] = B[:, G * t + g]
    return patA.astype(bf16), patB.astype(bf16)


def prep_inputs(inputs):
    """Host prep: per-core bf16 quarter-major xT + shared pattern matrices."""
    import ml_dtypes

    bf16 = ml_dtypes.bfloat16
    x = np.asarray(inputs["input_BI"], dtype=np.float32)
    As = [np.asarray(inputs[nm], dtype=np.float32) for nm in ("w0a", "w1a", "w2a")]
    Bs = [np.asarray(inputs[nm], dtype=np.float32) for nm in ("w0b", "w1b", "w2b")]
    patA, patB = _build_patterns(As, Bs, bf16)

    in_maps = []
    for c in range(N_CORES):
        xs = x[c * B_SHARD:(c + 1) * B_SHARD].T.astype(bf16)   # [4096, 512]
        im = {
            "patA": patA,
            "patB": patB,
            "xT": np.ascontiguousarray(
                xs.reshape(I_DIM, NQ, P).transpose(1, 0, 2)),  # [4, 4096, 128]
        }
        in_maps.append(im)
    return in_maps


def unshard_output(res_list, bias):
    """[2, 128, 32, 256] per core -> [4096, 4096] fp32 + bias."""
    outs = []
    for r in res_list:
        arr = np.asarray(r["y"], dtype=np.float32)   # [2, c, u, b]
        outs.append(arr.transpose(0, 3, 1, 2).reshape(B_SHARD, O_DIM))
    return np.concatenate(outs, axis=0) + bias[None, :]


def kernel(**inputs):
    from concourse.bass_utils import run_bass_kernel_spmd

    in_maps = prep_inputs(inputs)
    if "nc" not in _NC_CACHE:
        _NC_CACHE["nc"] = build_nc()
    res = run_bass_kernel_spmd(_NC_CACHE["nc"], in_maps,
                               core_ids=list(range(N_CORES)))
    bias = np.asarray(inputs["bias_O"], dtype=np.float32)
    return unshard_output([r for r in res.results], bias)
